# revision 4
# baseline (speedup 1.0000x reference)
"""Trainium2 Bass kernel for nn_AlpacaMoca_15109694948019.

Math: per (b,z,u) with A = Linv[b,z,u] (128x128), phi = encoder(x)[b]:
    t    = A^T phi                      (one matvec per matrix)
    mu   = t . Q[b,z,u]                 (= phi^T A Q)
    sig  = t . phi                      (= phi^T A phi, scalar symmetry)
    pred = exp(logSigEps[u]) * (1 + sig)

Sharding: batch (B=16) split 2 rows per core across 8 cores. Linv/Q are
fully independent along b; encoder weights + logSigEps replicated.

Layout: Linv streamed as [q=partition, (z,u,p)=free] tiles (contiguous
512B runs per partition, 2 MiB per DMA). Each matrix is the *stationary*
matmul operand; the moving operand is the single phi column, so t lands
as a PSUM column and 512 matrices fill one PSUM bank T[128, 512] per b.
Stage 2 reduces T against Q^T (PE-transposed on chip) and phi with DVE
elementwise ops + ones-vector matmul column sums.
"""

import numpy as np

B, Z, U, P, X, H = 16, 64, 8, 128, 64, 128
N_CORES = 8
B_PER = B // N_CORES          # 2 batch rows per core
J = Z * U                     # 512 matrices per batch row
ZT = 4                        # z rows per Linv DMA tile (4*8*64KiB = 2 MiB)
LINV_BUFS = 4                 # SBUF double-buffer depth for Linv tiles

_CACHE = {}


def _build_nc():
    import concourse.bacc as bacc
    import concourse.tile as tile
    from concourse import mybir
    from concourse.masks import make_identity

    f32 = mybir.dt.float32
    AF = mybir.ActivationFunctionType
    Alu = mybir.AluOpType

    nc = bacc.Bacc("TRN2")

    x_d = nc.dram_tensor("x", [B_PER, X], f32, kind="ExternalInput")
    linv_d = nc.dram_tensor("Linv", [B_PER, Z, U, P, P], f32, kind="ExternalInput")
    q_d = nc.dram_tensor("Q", [B_PER, Z, U, P], f32, kind="ExternalInput")
    w1_d = nc.dram_tensor("W1", [X, H], f32, kind="ExternalInput")
    b1_d = nc.dram_tensor("b1", [H], f32, kind="ExternalInput")
    w2_d = nc.dram_tensor("W2", [H, H], f32, kind="ExternalInput")
    b2_d = nc.dram_tensor("b2", [H], f32, kind="ExternalInput")
    w3_d = nc.dram_tensor("W3", [H, H], f32, kind="ExternalInput")
    b3_d = nc.dram_tensor("b3", [H], f32, kind="ExternalInput")
    w4_d = nc.dram_tensor("W4", [H, P], f32, kind="ExternalInput")
    b4_d = nc.dram_tensor("b4", [P], f32, kind="ExternalInput")
    lse_d = nc.dram_tensor("logSigEps", [U], f32, kind="ExternalInput")

    mu_d = nc.dram_tensor("mu", [B_PER, J], f32, kind="ExternalOutput")
    pred_d = nc.dram_tensor("pred", [B_PER, J], f32, kind="ExternalOutput")

    with tile.TileContext(nc) as tc:
        with (
            tc.tile_pool(name="const", bufs=1) as cpool,
            tc.tile_pool(name="lin", bufs=LINV_BUFS) as lpool,
            tc.tile_pool(name="work", bufs=2) as wpool,
            tc.tile_pool(name="qtp", bufs=2) as qpool,
            tc.tile_pool(name="outp", bufs=2) as opool,
            tc.tile_pool(name="encps", bufs=2, space="PSUM") as encps,
            tc.tile_pool(name="tps", bufs=2, space="PSUM") as tpool,
            tc.tile_pool(name="mvps", bufs=1, space="PSUM") as mvpool,
        ):
            # ---- constants ----
            ident = cpool.tile([128, 128], f32)
            make_identity(nc, ident[:])
            ones = cpool.tile([128, 1], f32)
            nc.gpsimd.memset(ones[:], 1.0)

            w1 = cpool.tile([X, H], f32)
            nc.sync.dma_start(w1[:], w1_d[:])
            w2 = cpool.tile([H, H], f32)
            nc.sync.dma_start(w2[:], w2_d[:])
            w3 = cpool.tile([H, H], f32)
            nc.sync.dma_start(w3[:], w3_d[:])
            w4 = cpool.tile([H, P], f32)
            nc.sync.dma_start(w4[:], w4_d[:])
            b1 = cpool.tile([H, 1], f32)
            nc.sync.dma_start(b1[:], b1_d[:, None])
            b2 = cpool.tile([H, 1], f32)
            nc.sync.dma_start(b2[:], b2_d[:, None])
            b3 = cpool.tile([H, 1], f32)
            nc.sync.dma_start(b3[:], b3_d[:, None])
            b4 = cpool.tile([P, 1], f32)
            nc.sync.dma_start(b4[:], b4_d[:, None])
            xT = cpool.tile([X, B_PER], f32)
            nc.sync.dma_start(xT[:], x_d[:].rearrange("b x -> x b"))
            lse = cpool.tile([1, U], f32)
            nc.sync.dma_start(lse[:], lse_d[None, :])

            # exp(logSigEps) replicated 64x along free dim -> [1, J] (u fastest)
            esig = cpool.tile([1, J], f32)
            nc.scalar.activation(esig[:, 0:U], lse[:], AF.Exp)
            n = U
            while n < J:
                m = min(n, J - n)
                nc.vector.tensor_copy(esig[:, n : n + m], esig[:, 0:m])
                n += m

            # ---- encoder: phi^T as [P, B_PER] ----
            def elu(h_ps, bias, out_sb):
                # out = elu(h_ps + bias) = relu(z) + exp(min(z,0)) - 1
                r = wpool.tile([H, B_PER], f32, tag="elu_r")
                nc.scalar.activation(r[:], h_ps[:], AF.Relu, bias=bias[:])
                zm = wpool.tile([H, B_PER], f32, tag="elu_z")
                nc.scalar.activation(zm[:], h_ps[:], AF.Identity, bias=bias[:])
                nc.vector.tensor_scalar_min(zm[:], zm[:], 0.0)
                nc.scalar.activation(zm[:], zm[:], AF.Exp)
                # out = (zm - 1) + r
                nc.vector.scalar_tensor_tensor(
                    out=out_sb[:], in0=zm[:], scalar=-1.0, in1=r[:],
                    op0=Alu.add, op1=Alu.add,
                )

            hp = encps.tile([H, B_PER], f32, tag="encmm")
            nc.tensor.matmul(hp[:], w1[:], xT[:])
            h1 = wpool.tile([H, B_PER], f32, tag="h")
            elu(hp, b1, h1)

            hp = encps.tile([H, B_PER], f32, tag="encmm")
            nc.tensor.matmul(hp[:], w2[:], h1[:])
            h2 = wpool.tile([H, B_PER], f32, tag="h")
            elu(hp, b2, h2)

            hp = encps.tile([H, B_PER], f32, tag="encmm")
            nc.tensor.matmul(hp[:], w3[:], h2[:])
            h3 = wpool.tile([H, B_PER], f32, tag="h")
            elu(hp, b3, h3)

            hp = encps.tile([P, B_PER], f32, tag="encmm")
            nc.tensor.matmul(hp[:], w4[:], h3[:])
            phi = cpool.tile([P, B_PER], f32)
            nc.scalar.activation(phi[:], hp[:], AF.Identity, bias=b4[:])

            # ---- Q^T per b: [P, J] via PE transpose of [128,128] blocks ----
            qTs = []
            for bi in range(B_PER):
                qT = qpool.tile([P, J], f32, tag="qT")
                qflat = q_d[bi].rearrange("z u p -> (z u) p")
                for j4 in range(J // 128):
                    qt_in = wpool.tile([128, P], f32, tag="qin")
                    nc.sync.dma_start(qt_in[:], qflat[j4 * 128 : (j4 + 1) * 128, :])
                    qt_ps = encps.tile([P, 128], f32, tag="qtps")
                    nc.tensor.transpose(qt_ps[:], qt_in[:], ident[:])
                    nc.vector.tensor_copy(qT[:, j4 * 128 : (j4 + 1) * 128], qt_ps[:])
                qTs.append(qT)

            # ---- main: per b, 512 stationary-matmuls then reductions ----
            for bi in range(B_PER):
                T_ps = tpool.tile([P, J], f32, tag="T")
                for zt in range(Z // ZT):
                    lt = lpool.tile([128, ZT * U, P], f32, tag="linv")
                    src = linv_d[bi, zt * ZT : (zt + 1) * ZT].rearrange(
                        "z u q p -> q (z u) p"
                    )
                    nc.sync.dma_start(lt[:], src)
                    for m in range(ZT * U):
                        jj = zt * ZT * U + m
                        nc.tensor.matmul(
                            T_ps[:, jj : jj + 1],
                            lt[:, m, :],
                            phi[:, bi : bi + 1],
                        )

                # V = T * phi (per-partition scalar), U = T * Q^T
                V_sb = wpool.tile([P, J], f32, tag="V")
                nc.vector.tensor_scalar_mul(V_sb[:], T_ps[:], phi[:, bi : bi + 1])
                U_sb = wpool.tile([P, J], f32, tag="Umat")
                nc.vector.tensor_tensor(U_sb[:], T_ps[:], qTs[bi][:], op=Alu.mult)

                mv_ps = mvpool.tile([1, 2 * J], f32, tag="mv")
                nc.tensor.matmul(mv_ps[:, 0:J], ones[:], U_sb[:])
                nc.tensor.matmul(mv_ps[:, J : 2 * J], ones[:], V_sb[:])

                mu_sb = opool.tile([1, J], f32, tag="mu")
                nc.vector.tensor_copy(mu_sb[:], mv_ps[:, 0:J])
                pr_sb = opool.tile([1, J], f32, tag="pr")
                # pred = esig * (1 + sig)
                nc.vector.tensor_scalar_add(pr_sb[:], mv_ps[:, J : 2 * J], 1.0)
                nc.vector.tensor_mul(pr_sb[:], pr_sb[:], esig[:])

                nc.sync.dma_start(mu_d[bi : bi + 1, :], mu_sb[:])
                nc.sync.dma_start(pred_d[bi : bi + 1, :], pr_sb[:])

    nc.finalize()
    return nc


def _get_nc():
    if "nc" not in _CACHE:
        _CACHE["nc"] = _build_nc()
    return _CACHE["nc"]


def _make_in_maps(inputs):
    x = np.ascontiguousarray(np.asarray(inputs["x"], dtype=np.float32))
    Linv = np.ascontiguousarray(np.asarray(inputs["Linv"], dtype=np.float32))
    Q2 = np.ascontiguousarray(np.asarray(inputs["Q"], dtype=np.float32)[:, :, :, 0, :])
    shared = {
        n: np.ascontiguousarray(np.asarray(inputs[n], np.float32))
        for n in ["W1", "b1", "W2", "b2", "W3", "b3", "W4", "b4", "logSigEps"]
    }
    in_maps = []
    for c in range(N_CORES):
        sl = slice(c * B_PER, (c + 1) * B_PER)
        in_maps.append({"x": x[sl], "Linv": Linv[sl], "Q": Q2[sl], **shared})
    return in_maps


def kernel(x, Linv, Q, W1, b1, W2, b2, W3, b3, W4, b4, logSigEps):
    from concourse.bass_utils import run_bass_kernel_spmd

    in_maps = _make_in_maps(dict(
        x=x, Linv=Linv, Q=Q, W1=W1, b1=b1, W2=W2, b2=b2, W3=W3, b3=b3,
        W4=W4, b4=b4, logSigEps=logSigEps,
    ))
    nc = _get_nc()
    res = run_bass_kernel_spmd(nc, in_maps, list(range(N_CORES))).results

    mu = np.concatenate([r["mu"] for r in res], axis=0).reshape(B, Z, U, 1)
    pred = np.concatenate([r["pred"] for r in res], axis=0).reshape(B, Z, U)
    return mu, pred


# revision 16
# speedup vs baseline: 1.9741x; 1.9741x over previous
"""Trainium2 Bass kernel for nn_AlpacaMoca_15109694948019.

Math: per (b,z,u) with A = Linv[b,z,u] (128x128), phi = encoder(x)[b]:
    t    = A^T phi                      (one matvec per matrix)
    mu   = t . Q[b,z,u]                 (= phi^T A Q)
    sig  = t . phi                      (= phi^T A phi, scalar symmetry)
    pred = exp(logSigEps[u]) * (1 + sig)

Sharding: batch (B=16) split 2 rows per core across 8 cores. Linv/Q are
fully independent along b; encoder weights + logSigEps replicated.

Layout: Linv streamed as [q=partition, (z,u,p)=free] tiles (contiguous
512B runs per partition, 2 MiB per DMA). Each matrix is the *stationary*
matmul operand; the moving operand is the single phi column, so t lands
as a PSUM column and 512 matrices fill one PSUM bank T[128, 512] per b.
Stage 2 reduces T against Q^T (PE-transposed on chip) and phi with DVE
elementwise ops + ones-vector matmul column sums.
"""

import numpy as np

B, Z, U, P, X, H = 16, 64, 8, 128, 64, 128
N_CORES = 8
B_PER = B // N_CORES          # 2 batch rows per core
J = Z * U                     # 512 matrices per batch row
ZT = 4                        # z rows per Linv DMA tile (4*8*64KiB = 2 MiB)
LINV_BUFS = 4                 # SBUF double-buffer depth for Linv tiles

_CACHE = {}


def _build_nc():
    import concourse.bacc as bacc
    import concourse.tile as tile
    from concourse import mybir
    from concourse.masks import make_identity

    f32 = mybir.dt.float32
    f32r = mybir.dt.float32r
    AF = mybir.ActivationFunctionType
    Alu = mybir.AluOpType

    nc = bacc.Bacc("TRN2")

    x_d = nc.dram_tensor("x", [B_PER, X], f32, kind="ExternalInput")
    # f32r so the stage-1 matmuls can consume DMA'd tiles directly (the BIR
    # verifier requires fp32r matmul operands to originate as fp32r; an
    # ExternalInput has no producer to round). Same bits as fp32 on host.
    linv_d = nc.dram_tensor("Linv", [B_PER, Z, U, P, P], f32r, kind="ExternalInput")
    q_d = nc.dram_tensor("Q", [B_PER, Z, U, P], f32, kind="ExternalInput")
    w1_d = nc.dram_tensor("W1", [X, H], f32, kind="ExternalInput")
    b1_d = nc.dram_tensor("b1", [H], f32, kind="ExternalInput")
    w2_d = nc.dram_tensor("W2", [H, H], f32, kind="ExternalInput")
    b2_d = nc.dram_tensor("b2", [H], f32, kind="ExternalInput")
    w3_d = nc.dram_tensor("W3", [H, H], f32, kind="ExternalInput")
    b3_d = nc.dram_tensor("b3", [H], f32, kind="ExternalInput")
    w4_d = nc.dram_tensor("W4", [H, P], f32, kind="ExternalInput")
    b4_d = nc.dram_tensor("b4", [P], f32, kind="ExternalInput")
    lse_d = nc.dram_tensor("logSigEps", [U], f32, kind="ExternalInput")

    mu_d = nc.dram_tensor("mu", [B_PER, J], f32, kind="ExternalOutput")
    pred_d = nc.dram_tensor("pred", [B_PER, J], f32, kind="ExternalOutput")

    with tile.TileContext(nc) as tc:
        with (
            tc.tile_pool(name="const", bufs=1) as cpool,
            tc.tile_pool(name="lin", bufs=LINV_BUFS) as lpool,
            tc.tile_pool(name="work", bufs=2) as wpool,
            tc.tile_pool(name="qtp", bufs=2) as qpool,
            tc.tile_pool(name="outp", bufs=2) as opool,
            tc.tile_pool(name="encps", bufs=1, space="PSUM") as encps,
            tc.tile_pool(name="tps", bufs=2, space="PSUM") as tpool,
            tc.tile_pool(name="mvps", bufs=1, space="PSUM") as mvpool,
        ):
            # ---- constants ----
            ident = cpool.tile([128, 128], f32)
            make_identity(nc, ident[:])
            ones = cpool.tile([128, 1], f32)
            nc.gpsimd.memset(ones[:], 1.0)

            w1 = cpool.tile([X, H], f32)
            nc.sync.dma_start(w1[:], w1_d[:])
            w2 = cpool.tile([H, H], f32)
            nc.sync.dma_start(w2[:], w2_d[:])
            w3 = cpool.tile([H, H], f32)
            nc.sync.dma_start(w3[:], w3_d[:])
            w4 = cpool.tile([H, P], f32)
            nc.sync.dma_start(w4[:], w4_d[:])
            b1 = cpool.tile([H, 1], f32)
            nc.sync.dma_start(b1[:], b1_d[:, None])
            b2 = cpool.tile([H, 1], f32)
            nc.sync.dma_start(b2[:], b2_d[:, None])
            b3 = cpool.tile([H, 1], f32)
            nc.sync.dma_start(b3[:], b3_d[:, None])
            b4 = cpool.tile([P, 1], f32)
            nc.sync.dma_start(b4[:], b4_d[:, None])
            xT = cpool.tile([X, B_PER], f32)
            nc.sync.dma_start(xT[:], x_d[:].rearrange("b x -> x b"))
            lse = cpool.tile([1, U], f32)
            nc.sync.dma_start(lse[:], lse_d[None, :])

            # exp(logSigEps) replicated 64x along free dim -> [1, J] (u fastest)
            esig = cpool.tile([1, J], f32)
            nc.scalar.activation(esig[:, 0:U], lse[:], AF.Exp)
            n = U
            while n < J:
                m = min(n, J - n)
                nc.vector.tensor_copy(esig[:, n : n + m], esig[:, 0:m])
                n += m

            # ---- encoder: phi^T as [P, B_PER] ----
            def elu(h_ps, bias, out_sb):
                # out = elu(h_ps + bias) = relu(z) + exp(min(z,0)) - 1
                r = wpool.tile([H, B_PER], f32, tag="elu_r")
                nc.scalar.activation(r[:], h_ps[:], AF.Relu, bias=bias[:])
                zm = wpool.tile([H, B_PER], f32, tag="elu_z")
                nc.scalar.activation(zm[:], h_ps[:], AF.Identity, bias=bias[:])
                nc.vector.tensor_scalar_min(zm[:], zm[:], 0.0)
                nc.scalar.activation(zm[:], zm[:], AF.Exp)
                # out = (zm - 1) + r
                nc.vector.scalar_tensor_tensor(
                    out=out_sb[:], in0=zm[:], scalar=-1.0, in1=r[:],
                    op0=Alu.add, op1=Alu.add,
                )

            hp = encps.tile([H, B_PER], f32, tag="encmm")
            nc.tensor.matmul(hp[:], w1[:], xT[:])
            h1 = wpool.tile([H, B_PER], f32, tag="h")
            elu(hp, b1, h1)

            hp = encps.tile([H, B_PER], f32, tag="encmm")
            nc.tensor.matmul(hp[:], w2[:], h1[:])
            h2 = wpool.tile([H, B_PER], f32, tag="h")
            elu(hp, b2, h2)

            hp = encps.tile([H, B_PER], f32, tag="encmm")
            nc.tensor.matmul(hp[:], w3[:], h2[:])
            h3 = wpool.tile([H, B_PER], f32, tag="h")
            elu(hp, b3, h3)

            hp = encps.tile([P, B_PER], f32, tag="encmm")
            nc.tensor.matmul(hp[:], w4[:], h3[:])
            # phi in f32r: stage-1 matmuls consume it as the moving operand.
            # fp32r matmuls need moving free dim >= 2, so store phi duplicated
            # as [P, B_PER, 2].
            phi = cpool.tile([P, B_PER, 2], f32r)
            nc.scalar.activation(phi[:, :, 0], hp[:], AF.Identity, bias=b4[:])
            nc.vector.tensor_copy(phi[:, :, 1], phi[:, :, 0])
            # f32 copy for DVE stage-2 use
            phi32 = cpool.tile([P, B_PER], f32)
            nc.vector.tensor_copy(phi32[:], phi[:, :, 0])

            # ---- Q^T per b: [P, J] via PE transpose of [128,128] blocks ----
            qTs = []
            for bi in range(B_PER):
                qT = qpool.tile([P, J], f32, tag="qT")
                qflat = q_d[bi].rearrange("z u p -> (z u) p")
                for j4 in range(J // 128):
                    qt_in = wpool.tile([128, P], f32, tag="qin")
                    nc.sync.dma_start(qt_in[:], qflat[j4 * 128 : (j4 + 1) * 128, :])
                    qt_ps = encps.tile([P, 128], f32, tag="qtps")
                    nc.tensor.transpose(qt_ps[:], qt_in[:], ident[:])
                    nc.vector.tensor_copy(qT[:, j4 * 128 : (j4 + 1) * 128], qt_ps[:])
                qTs.append(qT)

            # ---- main: per b, 512 stationary-matmuls then reductions ----
            for bi in range(B_PER):
                T_ps = tpool.tile([P, J, 2], f32, tag="T")
                for zt in range(Z // ZT):
                    lt = lpool.tile([128, ZT * U, P], f32r, tag="linv")
                    src = linv_d[bi, zt * ZT : (zt + 1) * ZT].rearrange(
                        "z u q p -> q (z u) p"
                    )
                    nc.sync.dma_start(lt[:], src)
                    for m in range(ZT * U):
                        jj = zt * ZT * U + m
                        nc.tensor.matmul(
                            T_ps[:, jj, :],
                            lt[:, m, :],
                            phi[:, bi, :],
                        )

                # V = T * phi (per-partition scalar), U = T * Q^T
                V_sb = wpool.tile([P, J], f32, tag="V")
                nc.vector.tensor_scalar_mul(V_sb[:], T_ps[:, :, 0], phi32[:, bi : bi + 1])
                U_sb = wpool.tile([P, J], f32, tag="Umat")
                nc.vector.tensor_tensor(U_sb[:], T_ps[:, :, 0], qTs[bi][:], op=Alu.mult)

                mv_ps = mvpool.tile([1, 2 * J], f32, tag="mv")
                nc.tensor.matmul(mv_ps[:, 0:J], ones[:], U_sb[:])
                nc.tensor.matmul(mv_ps[:, J : 2 * J], ones[:], V_sb[:])

                mu_sb = opool.tile([1, J], f32, tag="mu")
                nc.vector.tensor_copy(mu_sb[:], mv_ps[:, 0:J])
                pr_sb = opool.tile([1, J], f32, tag="pr")
                # pred = esig * (1 + sig)
                nc.vector.tensor_scalar_add(pr_sb[:], mv_ps[:, J : 2 * J], 1.0)
                nc.vector.tensor_mul(pr_sb[:], pr_sb[:], esig[:])

                nc.sync.dma_start(mu_d[bi : bi + 1, :], mu_sb[:])
                nc.sync.dma_start(pred_d[bi : bi + 1, :], pr_sb[:])

    nc.finalize()
    return nc


def _get_nc():
    if "nc" not in _CACHE:
        _CACHE["nc"] = _build_nc()
    return _CACHE["nc"]


def _make_in_maps(inputs):
    x = np.ascontiguousarray(np.asarray(inputs["x"], dtype=np.float32))
    Linv = np.ascontiguousarray(np.asarray(inputs["Linv"], dtype=np.float32))
    Q2 = np.ascontiguousarray(np.asarray(inputs["Q"], dtype=np.float32)[:, :, :, 0, :])
    shared = {
        n: np.ascontiguousarray(np.asarray(inputs[n], np.float32))
        for n in ["W1", "b1", "W2", "b2", "W3", "b3", "W4", "b4", "logSigEps"]
    }
    in_maps = []
    for c in range(N_CORES):
        sl = slice(c * B_PER, (c + 1) * B_PER)
        in_maps.append({"x": x[sl], "Linv": Linv[sl], "Q": Q2[sl], **shared})
    return in_maps


def kernel(x, Linv, Q, W1, b1, W2, b2, W3, b3, W4, b4, logSigEps):
    from concourse.bass_utils import run_bass_kernel_spmd

    in_maps = _make_in_maps(dict(
        x=x, Linv=Linv, Q=Q, W1=W1, b1=b1, W2=W2, b2=b2, W3=W3, b3=b3,
        W4=W4, b4=b4, logSigEps=logSigEps,
    ))
    nc = _get_nc()
    res = run_bass_kernel_spmd(nc, in_maps, list(range(N_CORES))).results

    mu = np.concatenate([r["mu"] for r in res], axis=0).reshape(B, Z, U, 1)
    pred = np.concatenate([r["pred"] for r in res], axis=0).reshape(B, Z, U)
    return mu, pred


# revision 17
# speedup vs baseline: 2.9616x; 1.5002x over previous
"""Trainium2 Bass kernel for nn_AlpacaMoca_15109694948019.

Math: per (b,z,u) with A = Linv[b,z,u] (128x128), phi = encoder(x)[b]:
    t    = A^T phi                      (one matvec per matrix)
    mu   = t . Q[b,z,u]                 (= phi^T A Q)
    sig  = t . phi                      (= phi^T A phi, scalar symmetry)
    pred = exp(logSigEps[u]) * (1 + sig)

Sharding: batch (B=16) split 2 rows per core across 8 cores. Linv/Q are
fully independent along b; encoder weights + logSigEps replicated.

Layout: Linv streamed as [q=partition, (z,u,p)=free] tiles (contiguous
512B runs per partition, 2 MiB per DMA). Each matrix is the *stationary*
matmul operand; the moving operand is the single phi column, so t lands
as a PSUM column and 512 matrices fill one PSUM bank T[128, 512] per b.
Stage 2 reduces T against Q^T (PE-transposed on chip) and phi with DVE
elementwise ops + ones-vector matmul column sums.
"""

import numpy as np

B, Z, U, P, X, H = 16, 64, 8, 128, 64, 128
N_CORES = 8
B_PER = B // N_CORES          # 2 batch rows per core
J = Z * U                     # 512 matrices per batch row
ZT = 8                        # z rows per Linv DMA tile (8*8*32KiB = 2 MiB fp16)
LINV_BUFS = 4                 # SBUF double-buffer depth for Linv tiles

_CACHE = {}


def _build_nc():
    import concourse.bacc as bacc
    import concourse.tile as tile
    from concourse import mybir
    from concourse.masks import make_identity

    f32 = mybir.dt.float32
    f16 = mybir.dt.float16
    AF = mybir.ActivationFunctionType
    Alu = mybir.AluOpType

    nc = bacc.Bacc("TRN2")

    x_d = nc.dram_tensor("x", [B_PER, X], f32, kind="ExternalInput")
    # Linv is pre-cast to fp16 on the host: halves the HBM stream (the
    # memory-bound term) and enables fast weight load on the PE. Values are
    # ~N(0, 0.02) so fp16 rounding costs ~3e-4 relative on the outputs.
    linv_d = nc.dram_tensor("Linv", [B_PER, Z, U, P, P], f16, kind="ExternalInput")
    q_d = nc.dram_tensor("Q", [B_PER, Z, U, P], f32, kind="ExternalInput")
    w1_d = nc.dram_tensor("W1", [X, H], f32, kind="ExternalInput")
    b1_d = nc.dram_tensor("b1", [H], f32, kind="ExternalInput")
    w2_d = nc.dram_tensor("W2", [H, H], f32, kind="ExternalInput")
    b2_d = nc.dram_tensor("b2", [H], f32, kind="ExternalInput")
    w3_d = nc.dram_tensor("W3", [H, H], f32, kind="ExternalInput")
    b3_d = nc.dram_tensor("b3", [H], f32, kind="ExternalInput")
    w4_d = nc.dram_tensor("W4", [H, P], f32, kind="ExternalInput")
    b4_d = nc.dram_tensor("b4", [P], f32, kind="ExternalInput")
    lse_d = nc.dram_tensor("logSigEps", [U], f32, kind="ExternalInput")

    mu_d = nc.dram_tensor("mu", [B_PER, J], f32, kind="ExternalOutput")
    pred_d = nc.dram_tensor("pred", [B_PER, J], f32, kind="ExternalOutput")

    with tile.TileContext(nc) as tc:
        with (
            tc.tile_pool(name="const", bufs=1) as cpool,
            tc.tile_pool(name="lin", bufs=LINV_BUFS) as lpool,
            tc.tile_pool(name="work", bufs=2) as wpool,
            tc.tile_pool(name="qtp", bufs=2) as qpool,
            tc.tile_pool(name="outp", bufs=2) as opool,
            tc.tile_pool(name="encps", bufs=2, space="PSUM") as encps,
            tc.tile_pool(name="tps", bufs=2, space="PSUM") as tpool,
            tc.tile_pool(name="mvps", bufs=1, space="PSUM") as mvpool,
        ):
            # ---- constants ----
            ident = cpool.tile([128, 128], f32)
            make_identity(nc, ident[:])
            ones = cpool.tile([128, 1], f32)
            nc.gpsimd.memset(ones[:], 1.0)

            w1 = cpool.tile([X, H], f32)
            nc.sync.dma_start(w1[:], w1_d[:])
            w2 = cpool.tile([H, H], f32)
            nc.sync.dma_start(w2[:], w2_d[:])
            w3 = cpool.tile([H, H], f32)
            nc.sync.dma_start(w3[:], w3_d[:])
            w4 = cpool.tile([H, P], f32)
            nc.sync.dma_start(w4[:], w4_d[:])
            b1 = cpool.tile([H, 1], f32)
            nc.sync.dma_start(b1[:], b1_d[:, None])
            b2 = cpool.tile([H, 1], f32)
            nc.sync.dma_start(b2[:], b2_d[:, None])
            b3 = cpool.tile([H, 1], f32)
            nc.sync.dma_start(b3[:], b3_d[:, None])
            b4 = cpool.tile([P, 1], f32)
            nc.sync.dma_start(b4[:], b4_d[:, None])
            xT = cpool.tile([X, B_PER], f32)
            nc.sync.dma_start(xT[:], x_d[:].rearrange("b x -> x b"))
            lse = cpool.tile([1, U], f32)
            nc.sync.dma_start(lse[:], lse_d[None, :])

            # exp(logSigEps) replicated 64x along free dim -> [1, J] (u fastest)
            esig = cpool.tile([1, J], f32)
            nc.scalar.activation(esig[:, 0:U], lse[:], AF.Exp)
            n = U
            while n < J:
                m = min(n, J - n)
                nc.vector.tensor_copy(esig[:, n : n + m], esig[:, 0:m])
                n += m

            # ---- encoder: phi^T as [P, B_PER] ----
            def elu(h_ps, bias, out_sb):
                # out = elu(h_ps + bias) = relu(z) + exp(min(z,0)) - 1
                r = wpool.tile([H, B_PER], f32, tag="elu_r")
                nc.scalar.activation(r[:], h_ps[:], AF.Relu, bias=bias[:])
                zm = wpool.tile([H, B_PER], f32, tag="elu_z")
                nc.scalar.activation(zm[:], h_ps[:], AF.Identity, bias=bias[:])
                nc.vector.tensor_scalar_min(zm[:], zm[:], 0.0)
                nc.scalar.activation(zm[:], zm[:], AF.Exp)
                # out = (zm - 1) + r
                nc.vector.scalar_tensor_tensor(
                    out=out_sb[:], in0=zm[:], scalar=-1.0, in1=r[:],
                    op0=Alu.add, op1=Alu.add,
                )

            hp = encps.tile([H, B_PER], f32, tag="encmm")
            nc.tensor.matmul(hp[:], w1[:], xT[:])
            h1 = wpool.tile([H, B_PER], f32, tag="h")
            elu(hp, b1, h1)

            hp = encps.tile([H, B_PER], f32, tag="encmm")
            nc.tensor.matmul(hp[:], w2[:], h1[:])
            h2 = wpool.tile([H, B_PER], f32, tag="h")
            elu(hp, b2, h2)

            hp = encps.tile([H, B_PER], f32, tag="encmm")
            nc.tensor.matmul(hp[:], w3[:], h2[:])
            h3 = wpool.tile([H, B_PER], f32, tag="h")
            elu(hp, b3, h3)

            hp = encps.tile([P, B_PER], f32, tag="encmm")
            nc.tensor.matmul(hp[:], w4[:], h3[:])
            # phi in fp16: stage-1 matmuls consume it as the moving operand
            # (dtype must match the fp16 stationary Linv tiles).
            phi = cpool.tile([P, B_PER], f16)
            nc.scalar.activation(phi[:], hp[:], AF.Identity, bias=b4[:])
            # f32 copy for DVE stage-2 use
            phi32 = cpool.tile([P, B_PER], f32)
            nc.vector.tensor_copy(phi32[:], phi[:])

            # ---- Q^T per b: [P, J] via PE transpose of [128,128] blocks ----
            qTs = []
            for bi in range(B_PER):
                qT = qpool.tile([P, J], f32, tag="qT")
                qflat = q_d[bi].rearrange("z u p -> (z u) p")
                for j4 in range(J // 128):
                    qt_in = wpool.tile([128, P], f32, tag="qin")
                    nc.sync.dma_start(qt_in[:], qflat[j4 * 128 : (j4 + 1) * 128, :])
                    qt_ps = encps.tile([P, 128], f32, tag="qtps")
                    nc.tensor.transpose(qt_ps[:], qt_in[:], ident[:])
                    nc.vector.tensor_copy(qT[:, j4 * 128 : (j4 + 1) * 128], qt_ps[:])
                qTs.append(qT)

            # ---- main: per b, 512 stationary-matmuls then reductions ----
            for bi in range(B_PER):
                T_ps = tpool.tile([P, J], f32, tag="T")
                for zt in range(Z // ZT):
                    lt = lpool.tile([128, ZT * U, P], f16, tag="linv")
                    src = linv_d[bi, zt * ZT : (zt + 1) * ZT].rearrange(
                        "z u q p -> q (z u) p"
                    )
                    nc.sync.dma_start(lt[:], src)
                    for m in range(ZT * U):
                        jj = zt * ZT * U + m
                        nc.tensor.matmul(
                            T_ps[:, jj : jj + 1],
                            lt[:, m, :],
                            phi[:, bi : bi + 1],
                        )

                # V = T * phi (per-partition scalar), U = T * Q^T
                V_sb = wpool.tile([P, J], f32, tag="V")
                nc.vector.tensor_scalar_mul(V_sb[:], T_ps[:], phi32[:, bi : bi + 1])
                U_sb = wpool.tile([P, J], f32, tag="Umat")
                nc.vector.tensor_tensor(U_sb[:], T_ps[:], qTs[bi][:], op=Alu.mult)

                mv_ps = mvpool.tile([1, 2 * J], f32, tag="mv")
                nc.tensor.matmul(mv_ps[:, 0:J], ones[:], U_sb[:])
                nc.tensor.matmul(mv_ps[:, J : 2 * J], ones[:], V_sb[:])

                mu_sb = opool.tile([1, J], f32, tag="mu")
                nc.vector.tensor_copy(mu_sb[:], mv_ps[:, 0:J])
                pr_sb = opool.tile([1, J], f32, tag="pr")
                # pred = esig * (1 + sig)
                nc.vector.tensor_scalar_add(pr_sb[:], mv_ps[:, J : 2 * J], 1.0)
                nc.vector.tensor_mul(pr_sb[:], pr_sb[:], esig[:])

                nc.sync.dma_start(mu_d[bi : bi + 1, :], mu_sb[:])
                nc.sync.dma_start(pred_d[bi : bi + 1, :], pr_sb[:])

    nc.finalize()
    return nc


def _get_nc():
    if "nc" not in _CACHE:
        _CACHE["nc"] = _build_nc()
    return _CACHE["nc"]


def _make_in_maps(inputs):
    x = np.ascontiguousarray(np.asarray(inputs["x"], dtype=np.float32))
    Linv = np.ascontiguousarray(np.asarray(inputs["Linv"], dtype=np.float32).astype(np.float16))
    Q2 = np.ascontiguousarray(np.asarray(inputs["Q"], dtype=np.float32)[:, :, :, 0, :])
    shared = {
        n: np.ascontiguousarray(np.asarray(inputs[n], np.float32))
        for n in ["W1", "b1", "W2", "b2", "W3", "b3", "W4", "b4", "logSigEps"]
    }
    in_maps = []
    for c in range(N_CORES):
        sl = slice(c * B_PER, (c + 1) * B_PER)
        in_maps.append({"x": x[sl], "Linv": Linv[sl], "Q": Q2[sl], **shared})
    return in_maps


def kernel(x, Linv, Q, W1, b1, W2, b2, W3, b3, W4, b4, logSigEps):
    from concourse.bass_utils import run_bass_kernel_spmd

    in_maps = _make_in_maps(dict(
        x=x, Linv=Linv, Q=Q, W1=W1, b1=b1, W2=W2, b2=b2, W3=W3, b3=b3,
        W4=W4, b4=b4, logSigEps=logSigEps,
    ))
    nc = _get_nc()
    res = run_bass_kernel_spmd(nc, in_maps, list(range(N_CORES))).results

    mu = np.concatenate([r["mu"] for r in res], axis=0).reshape(B, Z, U, 1)
    pred = np.concatenate([r["pred"] for r in res], axis=0).reshape(B, Z, U)
    return mu, pred
